# revision 24
# baseline (speedup 1.0000x reference)
"""Haar DWT2D (reflect-pad, stride-2 2x2) on Trainium2 via Bass/Tile.

Input  x: (8, 32, 512, 512) f32  ->  Output: (8, 128, 257, 257) f32.

Sharding: pure data parallel over the batch dim — core b handles x[b]
(32 independent 512x512 planes), no cross-core communication.

Math per plane (see reference): with xp = reflect-pad-1(x), window (i,j)
reads taps a=xp[2i,2j], b=xp[2i,2j+1], c=xp[2i+1,2j], d=xp[2i+1,2j+1]:
  LL=.5(a+b+c+d)  LH=.5(-a+b-c+d)  HL=.5(-a-b+c+d)  HH=.5(a-b-c+d)
Separable butterfly: row stage P=.5(u+v), M=.5(v-u) with u=xp[2i] (odd x
row), v=xp[2i+1] (even x row); col stage on even/odd columns of P/M.

Performance shape (measured on HW): DMA is descriptor-bound here (one
descriptor per SBUF partition per contiguous DRAM run, ~25-40ns each at
the generator), and a DMA spanning < 128 partitions only reaches half
the SDMA engines.  So the layout maximizes bytes/descriptor while
keeping every compute op near 128 partitions:

  Main pass, per plane: 126 partitions x 4 x-rows (rows 4q+1..4q+4,
  q=0..125) = windows 2q+1, 2q+2 -> one 1MB load DMA per plane with 8KB
  descriptors; the internal DRAM output layout (c, i, k, w) makes each
  partition's whole result (2 rows x 4 subbands) one contiguous 8.2KB
  run -> one 1MB store DMA per plane, 1 descriptor/partition.  The host
  transposes (c, i, k, w) -> (k, c, i, w) during the final gather.
  Tail pass: windows 253-255 (x rows 505..510) batched over all planes.
  Edge pass: windows 0 and 256 (x row pairs (0,1)/(510,511), u/v order
  reversed) batched over all planes.

Engine split: ACT halves the loaded tile (the only scale in the whole
butterfly; folding it into the DVE stage is blocked by the 1-sync-wait
S2S2D2_STT struct); DVE does P/M, reflect-mirror cols, and all four
subband combines.  Loads go on the sync HWDGE ring, stores on the
scalar ring so the two descriptor generators run concurrently.
"""

from contextlib import nullcontext

import numpy as np

import concourse.bacc as bacc
import concourse.mybir as mybir
from concourse.bass_utils import run_bass_kernel_spmd
from concourse.tile import TileContext

B = 8        # batch -> one core each
C = 32       # channels (planes) per core
H = W = 512
HO = WO = 257
F32 = mybir.dt.float32


def _emit_pass(nc, pool, ld, n, T, u_first, stores, ring, use_pool=True):
    """Butterfly for `n` partitions each holding T (u,v) x-row pairs laid
    out as 2T consecutive 512-wide rows in SBUF tile `ld` [n, 2T*512].
    stores: list of (p0, p1, dst_ap) with dst_ap shaped [p1-p0, 4, T, 257].
    """
    W2 = 2 * T * 512
    if use_pool == "dmaonly":
        junk = pool.tile([128, 4 * T * 257], F32, tag="out")
        nc.gpsimd.memset(junk[:], 0.0)
        for p0, p1, dst in stores:
            nc.scalar.dma_start(out=dst, in_=junk[p0:p1])
        return
    ldh = pool.tile([128, W2], F32, tag="ldh")
    nc.scalar.mul(ldh[:n], ld[:n, 0:W2], 0.5)
    ld3 = ldh[:n].rearrange("p (r w) -> p r w", w=512)  # [n, 2T, 512]
    u0, v0 = (0, 1) if u_first else (1, 0)
    usl = ld3[:, u0:2 * T:2, :]
    vsl = ld3[:, v0:2 * T:2, :]

    # pm: 2T sections of width 514 (T padded-P sections, then T padded-M)
    pm = pool.tile([128, 2 * T * 514], F32, tag="pm")
    pm3 = pm[:n].rearrange("p (s x) -> p s x", x=514)   # [n, 2T, 514]
    row_eng = nc.gpsimd if use_pool == "pm" else nc.vector
    row_eng.tensor_add(pm3[:, 0:T, 1:513], usl, vsl)
    row_eng.tensor_sub(pm3[:, T:2 * T, 1:513], vsl, usl)
    # reflect cols of every section in one op: col0 <- col2, col513 <- col511
    nc.vector.tensor_copy(pm3[:, :, 0:514:513], pm3[:, :, 2:512:509])

    # Merged col stage: ONE add over all 2T sections (P-sections -> LL,
    # M-sections -> HL) and ONE sub (-> LH, HH).  Per-partition output
    # order is (g in {sum,diff}, section, w): LL*, HL*, LH*, HH* with t
    # inside each subband; the host gather decodes this.
    out_t = pool.tile([128, 4 * T * 257], F32, tag="out")
    os3 = out_t[:n].rearrange("p (s w) -> p s w", w=257)
    ev, od = pm3[:, :, 0:514:2], pm3[:, :, 1:514:2]
    nc.vector.tensor_add(os3[:, 0:2 * T, :], ev, od)        # LL*, HL*
    eng = nc.gpsimd if use_pool else nc.vector
    eng.tensor_sub(os3[:, 2 * T:4 * T, :], od, ev)          # LH*, HH*

    for p0, p1, dst in stores:
        nc.scalar.dma_start(out=dst, in_=out_t[p0:p1])


def _build(loop_n=None, mode="full", bufs=4):
    """loop_n: if set, repeat the whole workload loop_n times inside one
    NEFF via a Tile For_i (benchmark amplification; output unchanged)."""
    use_pool = {"dmaonly": "dmaonly", "pool": True, "pmpool": "pm"}.get(mode, False)
    nc = bacc.Bacc("TRN2", debug=False, enable_asserts=False)
    x = nc.dram_tensor("x", [C, H, W], F32, kind="ExternalInput")
    y = nc.dram_tensor("y", [C, HO * 4 * WO], F32, kind="ExternalOutput")
    with TileContext(nc) as tc:
        loop_cm = tc.For_i(0, loop_n, 1) if loop_n else nullcontext()
        with loop_cm:
            with tc.tile_pool(name="p", bufs=bufs) as pool:
                # Main pass: windows 1..252 of each plane, one plane per
                # 126-partition block (2 windows per partition).
                for c in range(C):
                    ld = pool.tile([128, 2048], F32, tag="ld")
                    src = x[c, 1:505, :].rearrange("(q e) w -> q (e w)", e=4)
                    nc.sync.dma_start(out=ld[:126], in_=src)
                    dst = y[c, 0:126 * 2056].rearrange(
                        "(q s) -> q s", s=2056
                    )
                    _emit_pass(nc, pool, ld, 126, 2, True, [(0, 126, dst)],
                               c % 2, use_pool)
                # Tail pass: windows 253..255, all planes (x rows 505..510).
                ldt = pool.tile([32, 3072], F32, tag="ld")
                nc.sync.dma_start(
                    out=ldt[:],
                    in_=x[:, 505:511, :].rearrange("c r w -> c (r w)"),
                )
                dstt = y[:, 259056:259056 + 3084]
                _emit_pass(nc, pool, ldt, 32, 3, True, [(0, 32, dstt)], 0,
                           use_pool)
                # Edge pass: windows 0 and 256 (v-row comes first in memory).
                lde = pool.tile([64, 1024], F32, tag="ld")
                nc.sync.dma_start(
                    out=lde[0:32],
                    in_=x[:, 0:2, :].rearrange("c r w -> c (r w)"),
                )
                nc.sync.dma_start(
                    out=lde[32:64],
                    in_=x[:, 510:512, :].rearrange("c r w -> c (r w)"),
                )
                dst0 = y[:, 262140:262140 + 1028]
                dst1 = y[:, 263168:263168 + 1028]
                _emit_pass(nc, pool, lde, 64, 1, False,
                           [(0, 32, dst0), (32, 64, dst1)], 1, use_pool)
    nc.finalize()  # Bacc: register alloc + event-semaphore split (1 wait/inst)
    return nc


_NC = None


def _get_nc():
    global _NC
    if _NC is None:
        _NC = _build()
    return _NC


def _run(x, **spmd_kwargs):
    """x: (8, 32, 512, 512) f32 -> ((8, 128, 257, 257) f32, BassKernelResults)."""
    x = np.ascontiguousarray(np.asarray(x, dtype=np.float32))
    assert x.shape == (B, C, H, W), x.shape
    nc = _get_nc()
    in_maps = [{"x": np.ascontiguousarray(x[b])} for b in range(B)]
    res = run_bass_kernel_spmd(nc, in_maps, core_ids=list(range(B)), **spmd_kwargs)
    out = np.empty((B, 4, C, HO, WO), dtype=np.float32)
    # per-partition store order is (g, section, w): gk index g*2+m lists
    # subbands as [LL, HL, LH, HH]; final k order is [LL, LH, HL, HH].
    GK = (0, 2, 1, 3)
    for b in range(B):
        yb = res.results[b]["y"]
        main = yb[:, :259056].reshape(C, 126, 4, 2, 257)
        tail = yb[:, 259056:262140].reshape(C, 4, 3, 257)
        e0 = yb[:, 262140:263168].reshape(C, 4, 257)
        e1 = yb[:, 263168:264196].reshape(C, 4, 257)
        for k, gk in enumerate(GK):
            out[b, k, :, 1:253, :] = main[:, :, gk].reshape(C, 252, 257)
            out[b, k, :, 253:256, :] = tail[:, gk]
            out[b, k, :, 0, :] = e0[:, gk]
            out[b, k, :, 256, :] = e1[:, gk]
    return out.reshape(B, 4 * C, HO, WO), res


def kernel(x, filters=None, **_ignored):
    """Full-input entry point; `filters` is the fixed Haar bank (hardcoded)."""
    return _run(x)[0]


if __name__ == "__main__":
    rng = np.random.default_rng(0)
    xs = rng.standard_normal((B, C, H, W)).astype(np.float32)
    yv, _ = _run(xs)
    print(yv.shape, yv.dtype)


# revision 25
# speedup vs baseline: 1.0293x; 1.0293x over previous
"""Haar DWT2D (reflect-pad, stride-2 2x2) on Trainium2 via Bass/Tile.

Input  x: (8, 32, 512, 512) f32  ->  Output: (8, 128, 257, 257) f32.

Sharding: pure data parallel over the batch dim — core b handles x[b]
(32 independent 512x512 planes), no cross-core communication.

Math per plane (see reference): with xp = reflect-pad-1(x), window (i,j)
reads taps a=xp[2i,2j], b=xp[2i,2j+1], c=xp[2i+1,2j], d=xp[2i+1,2j+1]:
  LL=.5(a+b+c+d)  LH=.5(-a+b-c+d)  HL=.5(-a-b+c+d)  HH=.5(a-b-c+d)
Separable butterfly: row stage P=.5(u+v), M=.5(v-u) with u=xp[2i] (odd x
row), v=xp[2i+1] (even x row); col stage on even/odd columns of P/M.

Performance shape (measured on HW): DMA is descriptor-bound here (one
descriptor per SBUF partition per contiguous DRAM run, ~25-40ns each at
the generator), and a DMA spanning < 128 partitions only reaches half
the SDMA engines.  So the layout maximizes bytes/descriptor while
keeping every compute op near 128 partitions:

  Main pass, per plane: 126 partitions x 4 x-rows (rows 4q+1..4q+4,
  q=0..125) = windows 2q+1, 2q+2 -> one 1MB load DMA per plane with 8KB
  descriptors; the internal DRAM output layout (c, i, k, w) makes each
  partition's whole result (2 rows x 4 subbands) one contiguous 8.2KB
  run -> one 1MB store DMA per plane, 1 descriptor/partition.  The host
  transposes (c, i, k, w) -> (k, c, i, w) during the final gather.
  Tail pass: windows 253-255 (x rows 505..510) batched over all planes.
  Edge pass: windows 0 and 256 (x row pairs (0,1)/(510,511), u/v order
  reversed) batched over all planes.

Engine split: ACT halves the loaded tile (the only scale in the whole
butterfly; folding it into the DVE stage is blocked by the 1-sync-wait
S2S2D2_STT struct); DVE does P/M, reflect-mirror cols, and all four
subband combines.  Loads go on the sync HWDGE ring, stores on the
scalar ring so the two descriptor generators run concurrently.
"""

from contextlib import nullcontext

import numpy as np

import concourse.bacc as bacc
import concourse.mybir as mybir
from concourse.bass_utils import run_bass_kernel_spmd
from concourse.tile import TileContext

B = 8        # batch -> one core each
C = 32       # channels (planes) per core
H = W = 512
HO = WO = 257
F32 = mybir.dt.float32


def _emit_pass(nc, pool, ld, n, T, u_first, stores, ring, use_pool=True):
    """Butterfly for `n` partitions each holding T (u,v) x-row pairs laid
    out as 2T consecutive 512-wide rows in SBUF tile `ld` [n, 2T*512].
    stores: list of (p0, p1, dst_ap) with dst_ap shaped [p1-p0, 4, T, 257].
    """
    W2 = 2 * T * 512
    if use_pool == "dmaonly":
        junk = pool.tile([128, 4 * T * 257], F32, tag="out")
        nc.gpsimd.memset(junk[:], 0.0)
        for p0, p1, dst in stores:
            nc.scalar.dma_start(out=dst, in_=junk[p0:p1])
        return
    ldh = pool.tile([128, W2], F32, tag="ldh")
    nc.scalar.mul(ldh[:n], ld[:n, 0:W2], 0.5)
    ld3 = ldh[:n].rearrange("p (r w) -> p r w", w=512)  # [n, 2T, 512]
    u0, v0 = (0, 1) if u_first else (1, 0)
    usl = ld3[:, u0:2 * T:2, :]
    vsl = ld3[:, v0:2 * T:2, :]

    # pm: 2T sections of width 514 (T padded-P sections, then T padded-M)
    pm = pool.tile([128, 2 * T * 514], F32, tag="pm")
    pm3 = pm[:n].rearrange("p (s x) -> p s x", x=514)   # [n, 2T, 514]
    row_eng = nc.gpsimd if use_pool == "pm" else nc.vector
    row_eng.tensor_add(pm3[:, 0:T, 1:513], usl, vsl)
    row_eng.tensor_sub(pm3[:, T:2 * T, 1:513], vsl, usl)
    # reflect cols of every section in one op: col0 <- col2, col513 <- col511
    nc.vector.tensor_copy(pm3[:, :, 0:514:513], pm3[:, :, 2:512:509])

    # Merged col stage: ONE add over all 2T sections (P-sections -> LL,
    # M-sections -> HL) and ONE sub (-> LH, HH).  Per-partition output
    # order is (g in {sum,diff}, section, w): LL*, HL*, LH*, HH* with t
    # inside each subband; the host gather decodes this.
    out_t = pool.tile([128, 4 * T * 257], F32, tag="out")
    os3 = out_t[:n].rearrange("p (s w) -> p s w", w=257)
    ev, od = pm3[:, :, 0:514:2], pm3[:, :, 1:514:2]
    nc.vector.tensor_add(os3[:, 0:2 * T, :], ev, od)        # LL*, HL*
    eng = nc.gpsimd if use_pool else nc.vector
    eng.tensor_sub(os3[:, 2 * T:4 * T, :], od, ev)          # LH*, HH*

    for p0, p1, dst in stores:
        nc.scalar.dma_start(out=dst, in_=out_t[p0:p1])


def _build(loop_n=None, mode="full", bufs=4):
    """loop_n: if set, repeat the whole workload loop_n times inside one
    NEFF via a Tile For_i (benchmark amplification; output unchanged)."""
    use_pool = {"dmaonly": "dmaonly", "pool": True, "pmpool": "pm"}.get(mode, False)
    nc = bacc.Bacc("TRN2", debug=False, enable_asserts=False)
    x = nc.dram_tensor("x", [C, H, W], F32, kind="ExternalInput")
    y = nc.dram_tensor("y", [C, HO * 4 * WO], F32, kind="ExternalOutput")
    with TileContext(nc) as tc:
        loop_cm = tc.For_i(0, loop_n, 1) if loop_n else nullcontext()
        with loop_cm:
            with tc.tile_pool(name="p", bufs=bufs) as pool:
                # Main pass: windows 1..252 of each plane, one plane per
                # 126-partition block (2 windows per partition).
                for c in range(C):
                    ld = pool.tile([128, 2048], F32, tag="ld")
                    src = x[c, 1:505, :].rearrange("(q e) w -> q (e w)", e=4)
                    nc.sync.dma_start(out=ld[:126], in_=src)
                    dst = y[c, 0:126 * 2056].rearrange(
                        "(q s) -> q s", s=2056
                    )
                    _emit_pass(nc, pool, ld, 126, 2, True, [(0, 126, dst)],
                               c % 2, use_pool)
                # Tail pass: windows 253..255, all planes (x rows 505..510).
                ldt = pool.tile([32, 3072], F32, tag="ld")
                nc.sync.dma_start(
                    out=ldt[:],
                    in_=x[:, 505:511, :].rearrange("c r w -> c (r w)"),
                )
                dstt = y[:, 259056:259056 + 3084]
                _emit_pass(nc, pool, ldt, 32, 3, True, [(0, 32, dstt)], 0,
                           use_pool)
                # Edge pass: windows 0 and 256 (v-row comes first in memory).
                lde = pool.tile([64, 1024], F32, tag="ld")
                nc.sync.dma_start(
                    out=lde[0:32],
                    in_=x[:, 0:2, :].rearrange("c r w -> c (r w)"),
                )
                nc.sync.dma_start(
                    out=lde[32:64],
                    in_=x[:, 510:512, :].rearrange("c r w -> c (r w)"),
                )
                dst0 = y[:, 262140:262140 + 1028]
                dst1 = y[:, 263168:263168 + 1028]
                _emit_pass(nc, pool, lde, 64, 1, False,
                           [(0, 32, dst0), (32, 64, dst1)], 1, use_pool)
    nc.finalize()  # Bacc: register alloc + event-semaphore split (1 wait/inst)
    return nc


# per-partition store order is (g, section, w): gk index g*2+m lists
# subbands as [LL, HL, LH, HH]; final k order is [LL, LH, HL, HH].
_GK = (0, 2, 1, 3)


def _decode(yb, out):
    """yb: (C, 257*4*257) raw core output -> out: (4, C, HO, WO)."""
    main = yb[:, :259056].reshape(C, 126, 4, 2, 257)
    tail = yb[:, 259056:262140].reshape(C, 4, 3, 257)
    e0 = yb[:, 262140:263168].reshape(C, 4, 257)
    e1 = yb[:, 263168:264196].reshape(C, 4, 257)
    for k, gk in enumerate(_GK):
        out[k, :, 1:253, :] = main[:, :, gk].reshape(C, 252, 257)
        out[k, :, 253:256, :] = tail[:, gk]
        out[k, :, 0, :] = e0[:, gk]
        out[k, :, 256, :] = e1[:, gk]


_NC = None


def _get_nc():
    global _NC
    if _NC is None:
        _NC = _build()
    return _NC


def _run(x, **spmd_kwargs):
    """x: (8, 32, 512, 512) f32 -> ((8, 128, 257, 257) f32, BassKernelResults)."""
    x = np.ascontiguousarray(np.asarray(x, dtype=np.float32))
    assert x.shape == (B, C, H, W), x.shape
    nc = _get_nc()
    in_maps = [{"x": np.ascontiguousarray(x[b])} for b in range(B)]
    res = run_bass_kernel_spmd(nc, in_maps, core_ids=list(range(B)), **spmd_kwargs)
    out = np.empty((B, 4, C, HO, WO), dtype=np.float32)
    for b in range(B):
        _decode(res.results[b]["y"], out[b])
    return out.reshape(B, 4 * C, HO, WO), res


def kernel(x, filters=None, **_ignored):
    """Full-input entry point; `filters` is the fixed Haar bank (hardcoded)."""
    return _run(x)[0]


if __name__ == "__main__":
    rng = np.random.default_rng(0)
    xs = rng.standard_normal((B, C, H, W)).astype(np.float32)
    yv, _ = _run(xs)
    print(yv.shape, yv.dtype)
